# revision 2
# baseline (speedup 1.0000x reference)
"""Conditionally-modulated 3x3 conv via F(2,3) Winograd along H, on 8 TRN2 cores.

Reference (per sample s):
    out[s] = conv2d(input[s] * cond[s], weight / sqrt(C*9)) + bias_mat[s]

Strategy: data-parallel over batch (2 samples/core). Along H, use Winograd
F(2,3): for each pair of output rows (2ty, 2ty+1), four transformed input
planes V0..V3 (row add/subs) feed four independent GEMM accumulations
M0..M3 (contracting 256 ic x 3 kx = 6 matmuls each, N=512), and the two
output rows are A^T combos: out0 = M0+M1+M2, out1 = M1-M2-M3. This cuts
PE column-streams per output from 18 to 12 (1.5x) vs direct conv.

All GEMM operands are bf16 (PE column rate is dtype-independent at/below
bf16, but Winograd needs fewer columns). W direction stays direct (3 kx
shifts of padded rows) so outputs are dense full rows. The per-sample cond
scale folds into the weights on device; the per-sample bias enters via the
fused scalar of the last A^T combos (scalar_tensor_tensor), so each output
row gets it exactly once.

Engine split per core: PE ~330us (1536 matmuls, N=512); ACT evicts PSUM as
one bf16 Copy per group (~120us; Identity-with-bias would need an act-table
load that faults on this axon terminal, Copy is table-free); DVE does the
V transform + A^T combos in bf16 (~200us). Output rows land in DRAM
deinterleaved ([.., 2, H/2, W]) so every DMA writes contiguous rows; the
host re-interleaves for free. End-to-end rel err ~5e-3 (bf16).
"""

import math

import numpy as np
import ml_dtypes

import concourse.mybir as mybir
import concourse.tile as tile
from concourse import bacc
from concourse.bass_utils import run_bass_kernel_spmd

B, C, H, W = 16, 256, 128, 128
NCORES = 8
B_LOC = B // NCORES  # samples per core
KH = KW = 3
NPT = 4  # Winograd points per row-pair
SLAB_TY = 16  # ty tiles per slab (32 output rows)
NSLAB = (H // 2) // SLAB_TY
SUB_TY = 4  # ty tiles per PSUM group (N = 4*128 = 512)
NSUB = SLAB_TY // SUB_TY
F32 = mybir.dt.float32
BF16 = mybir.dt.bfloat16
BF16_NP = ml_dtypes.bfloat16

EVICT_MODE = "act2"  # ACT Copy eviction (table-free); Identity-with-bias faults on this axon terminal

_cache = {}


def _build_one():
    """Single-device build for CoreSim debugging."""
    global NCORES
    saved = NCORES
    try:
        NCORES = 1
        return _build()
    finally:
        NCORES = saved


def _build(reps=1):
    """Build the per-core kernel. reps>1 (or "dyn") wraps the compute in a
    hardware loop for wall-clock differencing (axon dispatch ~100ms)."""
    dyn = reps == "dyn"
    nc = bacc.Bacc("TRN2", target_bir_lowering=False, debug=False, num_devices=NCORES)

    x_d = nc.dram_tensor("x", [B_LOC, C, H, W], BF16, kind="ExternalInput").ap()
    # w[p, icb, pt, kx, oc] = U[pt, oc, icb*128+p, kx]  (host-transformed)
    w_d = nc.dram_tensor("w", [128, 2, NPT, KW, C], BF16, kind="ExternalInput").ap()
    # cw[p, s, 0:2] = cond per ic block; [2+ocb] = +bias; [4+ocb] = -bias
    cw_d = nc.dram_tensor("cw", [128, B_LOC, 6], F32, kind="ExternalInput").ap()
    if dyn:
        r_d = nc.dram_tensor("r", [1, 1], mybir.dt.uint32, kind="ExternalInput").ap()
    y_d = nc.dram_tensor("y", [B_LOC, C, 2, H // 2, W], BF16, kind="ExternalOutput").ap()

    with tile.TileContext(nc) as tc:
        with (
            tc.tile_pool(name="const", bufs=1) as const_pool,
            tc.tile_pool(name="wsp", bufs=2) as ws_pool,
            tc.tile_pool(name="mp", bufs=4) as m_pool,
            tc.tile_pool(name="op", bufs=6) as o_pool,
            tc.tile_pool(name="ps", bufs=2, space="PSUM") as ps_pool,
        ):
            w_base = const_pool.tile([128, 2, NPT, KW, C], BF16)
            nc.sync.dma_start(w_base[:], w_d[:])
            cw = const_pool.tile([128, B_LOC, 6], F32)
            nc.sync.dma_start(cw[:], cw_d[:])

            # Persistent double-buffered padded-input slabs [icb, 34 rows, W+2]
            # and Winograd-domain slabs [icb, pt, 16 ty, W+2]. d border columns
            # are zeroed once (interior DMAs never touch them); V borders stay
            # zero automatically (0 +/- 0 from the transform).
            d_bufs = [
                const_pool.tile([128, 2, 2 * SLAB_TY + 2, W + 2], BF16, name=f"d{i}")
                for i in range(2)
            ]
            v_bufs = [
                const_pool.tile([128, 2, NPT, SLAB_TY, W + 2], BF16, name=f"v{i}")
                for i in range(2)
            ]
            for db in d_bufs:
                nc.vector.memset(db[:, :, :, 0:1], 0)
                nc.vector.memset(db[:, :, :, W + 1 : W + 2], 0)

            import contextlib

            if dyn:
                r_sb = const_pool.tile([1, 1], mybir.dt.uint32)
                nc.sync.dma_start(r_sb[:], r_d[:])
                with tc.tile_critical():
                    n_iter = nc.values_load(
                        r_sb[0:1, 0:1],
                        min_val=0,
                        max_val=1 << 20,
                        skip_runtime_bounds_check=True,
                    )
                loop_cm = tc.For_i(0, n_iter, 1)
            elif reps > 1:
                loop_cm = tc.For_i(0, reps, 1)
            else:
                loop_cm = contextlib.nullcontext()
            with loop_cm:
                _emit_compute(
                    nc, tc, ws_pool, m_pool, o_pool, ps_pool,
                    x_d, y_d, cw, w_base, d_bufs, v_bufs,
                )

    nc.compile()
    return nc


def _emit_compute(nc, tc, ws_pool, m_pool, o_pool, ps_pool, x_d, y_d, cw, w_base, d_bufs, v_bufs):
    Identity = mybir.ActivationFunctionType.Identity
    Copy = mybir.ActivationFunctionType.Copy
    nrow = 2 * SLAB_TY + 2  # 34 padded rows per slab

    for s in range(B_LOC):
        # fold this sample's condition scale into the transformed weights
        w_s = ws_pool.tile([128, 2, NPT, KW, C], BF16, name="w_s")
        for icb in range(2):
            nc.vector.tensor_scalar_mul(
                w_s[:, icb], w_base[:, icb], cw[:, s, icb : icb + 1]
            )

        for k in range(NSLAB):
            buf = (s * NSLAB + k) % 2
            d = d_bufs[buf]
            v = v_bufs[buf]
            # padded rows l in [0, 33] <-> input rows 32k-1+l
            in_lo = max(32 * k - 1, 0)
            in_hi = min(32 * k + nrow - 1, H)
            dst_lo = in_lo - (32 * k - 1)
            nrows = in_hi - in_lo
            for icb in range(2):
                nc.sync.dma_start(
                    d[:, icb, dst_lo : dst_lo + nrows, 1 : W + 1],
                    x_d[s, icb * 128 : (icb + 1) * 128, in_lo:in_hi, :],
                )
            if k == 0:
                nc.vector.memset(d[:, :, 0:1, :], 0)
            if k == NSLAB - 1:
                nc.vector.memset(d[:, :, nrow - 1 : nrow, :], 0)

            # V transform: per ty, rows (2t..2t+3) -> 4 Winograd planes
            for icb in range(2):
                nc.vector.tensor_sub(
                    v[:, icb, 0], d[:, icb, 0 : nrow - 2 : 2], d[:, icb, 2:nrow:2]
                )
                nc.vector.tensor_add(
                    v[:, icb, 1], d[:, icb, 1 : nrow - 1 : 2], d[:, icb, 2:nrow:2]
                )
                nc.vector.tensor_sub(
                    v[:, icb, 2], d[:, icb, 2:nrow:2], d[:, icb, 1 : nrow - 1 : 2]
                )
                nc.vector.tensor_sub(
                    v[:, icb, 3], d[:, icb, 1 : nrow - 1 : 2], d[:, icb, 3:nrow:2]
                )

            for sub in range(NSUB):
                t0 = SUB_TY * sub
                for ocb in range(2):
                    ps = ps_pool.tile([128, NPT, SUB_TY * W], F32, name="ps")
                    for pt in range(NPT):
                        t = 0
                        for icb in range(2):
                            for kx in range(KW):
                                nc.tensor.matmul(
                                    ps[:, pt],
                                    w_s[:, icb, pt, kx, ocb * 128 : (ocb + 1) * 128],
                                    v[:, icb, pt, t0 : t0 + SUB_TY, kx : kx + W],
                                    start=(t == 0),
                                    stop=(t == 5),
                                )
                                t += 1
                    # PSUM -> SBUF bf16 eviction with bias folded in:
                    # m0 += b, m3 -= b  =>  out0 = m0+m1+m2+b, out1 = m1-m2-m3+b
                    m = m_pool.tile([128, NPT, SUB_TY * W], BF16, name="m")
                    if EVICT_MODE == "act":
                        nc.scalar.activation(
                            m[:, 0], ps[:, 0], Identity,
                            bias=cw[:, s, 2 + ocb : 3 + ocb],
                        )
                        nc.scalar.activation(m[:, 1:3], ps[:, 1:3], Copy)
                        nc.scalar.activation(
                            m[:, 3], ps[:, 3], Identity,
                            bias=cw[:, s, 4 + ocb : 5 + ocb],
                        )
                    elif EVICT_MODE == "act2":
                        nc.scalar.activation(m[:], ps[:], Copy)
                    else:
                        nc.vector.tensor_scalar_add(
                            m[:, 0], ps[:, 0], cw[:, s, 2 + ocb : 3 + ocb]
                        )
                        nc.vector.tensor_copy(m[:, 1:3], ps[:, 1:3])
                        nc.vector.tensor_scalar_add(
                            m[:, 3], ps[:, 3], cw[:, s, 4 + ocb : 5 + ocb]
                        )
                    # A^T combos (DVE, bf16 2x mode)
                    tt0 = o_pool.tile([128, SUB_TY, W], BF16, name="tt0")
                    o0 = o_pool.tile([128, SUB_TY, W], BF16, name="o0")
                    tt1 = o_pool.tile([128, SUB_TY, W], BF16, name="tt1")
                    o1 = o_pool.tile([128, SUB_TY, W], BF16, name="o1")
                    Alu = mybir.AluOpType
                    if EVICT_MODE == "act2":
                        # bias enters via the fused scalar in the last combos
                        nc.vector.tensor_add(tt0[:], m[:, 0], m[:, 1])
                        nc.vector.scalar_tensor_tensor(
                            o0[:], tt0[:], cw[:, s, 2 + ocb : 3 + ocb], m[:, 2],
                            op0=Alu.add, op1=Alu.add,
                        )
                        nc.vector.tensor_sub(tt1[:], m[:, 1], m[:, 2])
                        nc.vector.scalar_tensor_tensor(
                            o1[:], tt1[:], cw[:, s, 2 + ocb : 3 + ocb], m[:, 3],
                            op0=Alu.add, op1=Alu.subtract,
                        )
                    else:
                        nc.vector.tensor_add(tt0[:], m[:, 0], m[:, 1])
                        nc.vector.tensor_add(o0[:], tt0[:], m[:, 2])
                        nc.vector.tensor_sub(tt1[:], m[:, 1], m[:, 2])
                        nc.vector.tensor_sub(o1[:], tt1[:], m[:, 3])
                    ty0 = SLAB_TY * k + t0
                    oc0 = ocb * 128
                    nc.sync.dma_start(
                        y_d[s, oc0 : oc0 + 128, 0, ty0 : ty0 + SUB_TY, :], o0[:]
                    )
                    nc.sync.dma_start(
                        y_d[s, oc0 : oc0 + 128, 1, ty0 : ty0 + SUB_TY, :], o1[:]
                    )


def _get_nc():
    if "nc" not in _cache:
        _cache["nc"] = _build()
    return _cache["nc"]


def _make_in_maps(inputs):
    x = np.asarray(inputs["input"], dtype=np.float32)
    cond = np.asarray(inputs["condition_feature"], dtype=np.float32).reshape(B, C)
    weight = np.asarray(inputs["weight"], dtype=np.float32)
    bias = np.asarray(inputs["bias"], dtype=np.float32)

    scale = 1.0 / math.sqrt(C * KH * KW)
    g = weight * scale  # [oc, ic, ky, kx]
    U = np.stack(
        [
            g[:, :, 0, :],
            (g[:, :, 0, :] + g[:, :, 1, :] + g[:, :, 2, :]) * 0.5,
            (g[:, :, 0, :] - g[:, :, 1, :] + g[:, :, 2, :]) * 0.5,
            g[:, :, 2, :],
        ],
        axis=0,
    )  # [pt, oc, ic, kx]
    # -> [p, icb, pt, kx, oc]
    w_host = np.ascontiguousarray(
        U.transpose(2, 0, 3, 1)  # [ic, pt, kx, oc]
        .reshape(2, 128, NPT, KW, C)
        .transpose(1, 0, 2, 3, 4)
    ).astype(BF16_NP)
    bias_mat = np.repeat(bias, B).reshape(B, C)  # [s, oc]
    x_bf = np.ascontiguousarray(x).astype(BF16_NP)

    in_maps = []
    for c in range(NCORES):
        sl = slice(c * B_LOC, (c + 1) * B_LOC)
        cond_c = cond[sl]
        bias_c = bias_mat[sl]
        cw = np.empty((128, B_LOC, 6), dtype=np.float32)
        for s in range(B_LOC):
            cw[:, s, 0] = cond_c[s, 0:128]
            cw[:, s, 1] = cond_c[s, 128:256]
            cw[:, s, 2] = bias_c[s, 0:128]
            cw[:, s, 3] = bias_c[s, 128:256]
            cw[:, s, 4] = -bias_c[s, 0:128]
            cw[:, s, 5] = -bias_c[s, 128:256]
        in_maps.append({"x": x_bf[sl], "w": w_host, "cw": cw})
    return in_maps


def kernel(input, condition_feature, weight, bias):
    in_maps = _make_in_maps(
        {
            "input": input,
            "condition_feature": condition_feature,
            "weight": weight,
            "bias": bias,
        }
    )
    nc = _get_nc()
    res = run_bass_kernel_spmd(nc, in_maps, list(range(NCORES)))
    y = np.concatenate(
        [res.results[c]["y"].astype(np.float32) for c in range(NCORES)], axis=0
    )  # [B, C, 2, H//2, W]
    out = np.empty((B, C, H, W), dtype=np.float32)
    out[:, :, 0::2, :] = y[:, :, 0]
    out[:, :, 1::2, :] = y[:, :, 1]
    return out


if __name__ == "__main__":
    rng = np.random.default_rng(0)
    inputs = {
        "input": rng.standard_normal((B, C, H, W), dtype=np.float32),
        "condition_feature": rng.random((B, 1, C, 1, 1), dtype=np.float32),
        "weight": rng.standard_normal((C, C, KH, KW), dtype=np.float32),
        "bias": rng.standard_normal((C,), dtype=np.float32) * 0.1,
    }
    out = kernel(**inputs)
    print("out", out.shape, out.dtype, float(np.abs(out).max()))
